# revision 1
# baseline (speedup 1.0000x reference)
"""AdapterBlock3D kernel for 8 Trainium2 NeuronCores.

Strategy: the block is fully data-parallel over attention windows.
x is (2,16,16,16,768) with window size 8 -> 16 windows of 512 tokens
(no padding needed). LayerNorm / MLP / adapters are per-token, attention
is per-window, so each of the 8 cores independently processes 2 windows
(1024 tokens) end-to-end. Window partition/unpartition (pure data
movement) happens on the host; all FLOPs run on the NeuronCores.
"""

import numpy as np

DIM = 768
HEADS = 12
HD = DIM // HEADS
WS = 8
N = WS ** 3  # 512 tokens per window
SCALE = HD ** -0.5
ADAPT_SCALE = 0.5
N_CORES = 8

_compiled = {}


def _win_partition_np(x):
    # (B, 16,16,16, C) -> (B*8, 512, C), matching reference token order
    B, D, H, W, C = x.shape
    x = x.reshape(B, D // WS, WS, H // WS, WS, W // WS, WS, C)
    x = x.transpose(0, 1, 3, 5, 2, 4, 6, 7)
    return np.ascontiguousarray(x.reshape(-1, N, C))


def _win_unpartition_np(win, B, D, H, W):
    C = win.shape[-1]
    x = win.reshape(B, D // WS, H // WS, W // WS, WS, WS, WS, C)
    x = x.transpose(0, 1, 4, 2, 5, 3, 6, 7)
    return np.ascontiguousarray(x.reshape(B, D, H, W, C))


def _block_fn():
    import jax
    import jax.numpy as jnp

    def _ln(x, g, b, eps=1e-5):
        m = x.mean(-1, keepdims=True)
        v = ((x - m) ** 2).mean(-1, keepdims=True)
        return (x - m) * jax.lax.rsqrt(v + eps) * g + b

    def _rel(rel_pos):
        idx = jnp.arange(WS)[:, None] - jnp.arange(WS)[None, :] + (WS - 1)
        return rel_pos[idx]  # (WS, WS, HD)

    def f(x, ln1_g, ln1_b, qkv_w, qkv_b, rpd, rph, rpw, proj_w, proj_b,
          aa1_w, aa1_b, aa2_w, aa2_b, ln2_g, ln2_b,
          mlp1_w, mlp1_b, mlp2_w, mlp2_b, ma1_w, ma1_b, ma2_w, ma2_b):
        # x: (Bw_local, N, DIM) — this core's windows
        Bw = x.shape[0]
        shortcut = x
        h = _ln(x, ln1_g, ln1_b)
        qkv = (h.reshape(Bw * N, DIM) @ qkv_w.T + qkv_b)
        qkv = qkv.reshape(Bw, N, 3, HEADS, HD).transpose(2, 0, 3, 1, 4)
        qkv = qkv.reshape(3, Bw * HEADS, N, HD)
        q, k, v = qkv[0], qkv[1], qkv[2]
        attn = jnp.einsum('bqc,bkc->bqk', q * SCALE, k)
        rq = q.reshape(-1, WS, WS, WS, HD)
        rel_d = jnp.einsum('bdhwc,dkc->bdhwk', rq, _rel(rpd))
        rel_h = jnp.einsum('bdhwc,hkc->bdhwk', rq, _rel(rph))
        rel_w = jnp.einsum('bdhwc,wkc->bdhwk', rq, _rel(rpw))
        attn = (attn.reshape(-1, WS, WS, WS, WS, WS, WS)
                + rel_d[:, :, :, :, :, None, None]
                + rel_h[:, :, :, :, None, :, None]
                + rel_w[:, :, :, :, None, None, :]).reshape(-1, N, N)
        attn = jax.nn.softmax(attn, axis=-1)
        out = jnp.einsum('bqk,bkc->bqc', attn, v)
        out = out.reshape(Bw, HEADS, WS, WS, WS, HD)
        out = out.transpose(0, 2, 3, 4, 1, 5).reshape(Bw, N, DIM)
        out = out @ proj_w.T + proj_b
        # attention adapter (skip=True)
        out = out + (jax.nn.gelu(out @ aa1_w.T + aa1_b, approximate=False)
                     @ aa2_w.T + aa2_b)
        h2 = shortcut + out
        hn = _ln(h2, ln2_g, ln2_b)
        mlp = jax.nn.gelu(hn @ mlp1_w.T + mlp1_b, approximate=False) @ mlp2_w.T + mlp2_b
        ad = jax.nn.gelu(hn @ ma1_w.T + ma1_b, approximate=False) @ ma2_w.T + ma2_b
        return h2 + mlp + ADAPT_SCALE * ad

    return f


_W_NAMES = ['ln1_g', 'ln1_b', 'qkv_w', 'qkv_b', 'rel_pos_d', 'rel_pos_h',
            'rel_pos_w', 'proj_w', 'proj_b', 'aa1_w', 'aa1_b', 'aa2_w',
            'aa2_b', 'ln2_g', 'ln2_b', 'mlp1_w', 'mlp1_b', 'mlp2_w',
            'mlp2_b', 'ma1_w', 'ma1_b', 'ma2_w', 'ma2_b']


def kernel(**inputs):
    import jax

    x = np.asarray(inputs['x'], dtype=np.float32)
    B, D, H, W, C = x.shape
    win = _win_partition_np(x)                      # (16, 512, 768)
    n_win = win.shape[0]
    per = n_win // N_CORES
    shards = win.reshape(N_CORES, per, N, C)        # (8, 2, 512, 768)

    weights = [np.asarray(inputs[k], dtype=np.float32) for k in _W_NAMES]

    if 'fn' not in _compiled:
        f = _block_fn()
        _compiled['fn'] = jax.pmap(
            f, in_axes=(0,) + (None,) * len(_W_NAMES),
            devices=jax.devices()[:N_CORES])
    out = _compiled['fn'](shards, *weights)
    out = np.asarray(out).reshape(n_win, N, C)
    return _win_unpartition_np(out, B, D, H, W).astype(np.float32)

